# revision 1
# baseline (speedup 1.0000x reference)
"""Bandpass biquad filter (lowpass 200Hz - highpass 5kHz) as a Trainium2 kernel.

Strategy: the cascade of two biquads reduces to y = (h_lp - h_hp) * x, an IIR
whose impulse response decays below fp32 noise after ~640 taps (dominant pole
radius 0.980).  We evaluate it as an exact-FIR block-Toeplitz convolution on
the TensorEngine:

  y_T[f, c] = sum_d T_d @ x_T[:, c-d],   T_d[f, f'] = h[128*d + f - f']

with the audio in a transposed [time-within-block=partition, block=free]
layout (obtained for free via the 2-byte xbar transpose DMA).  fp32 precision
is kept by splitting x and h into fp16 hi+lo parts (products are then exact
in the PE's fp32 accumulator) and accumulating the cross terms in PSUM; tap
blocks d>=2 only need the hi*hi term because |h| has decayed by then.

Sharding: data-parallel, 64 (batch,channel) series over 8 cores (8 each).
"""

import numpy as np
import ml_dtypes  # noqa: F401  (fp16 used via numpy)

import concourse.bass as bass
import concourse.tile as tile
import concourse.mybir as mybir
from concourse import bacc

P = 128          # block size == PE contraction size
D = 5            # tap blocks: K = 640 taps
HIST = 16        # history columns kept in x_T tiles (multiple of 16, >= D-1)
S = 8            # series per core
NCORES = 8
T = 220500
NB = 1792        # padded blocks per series (1792*128 = 229376 >= 220500)
TPAD = NB * P
GROUPS = [512, 512, 512, 256]   # block-columns per matmul group (sum == NB)

QF = 0.707       # torchaudio default Q

_CACHE = {}


def _biquad_coeffs(kind, sr, cutoff):
    # Reference computes coefficients in float32 (jnp default); mimic exactly,
    # then promote to float64 for the impulse-response recursion.
    f32 = np.float32
    sr = f32(float(sr))
    cutoff = f32(float(cutoff))
    w0 = f32(2.0) * f32(np.pi) * cutoff / sr
    cos_w0 = np.cos(w0, dtype=f32)
    alpha = np.sin(w0, dtype=f32) / (f32(2.0) * f32(QF))
    if kind == "lp":
        b0 = (f32(1.0) - cos_w0) / f32(2.0)
        b1 = f32(1.0) - cos_w0
    else:
        b0 = (f32(1.0) + cos_w0) / f32(2.0)
        b1 = -(f32(1.0) + cos_w0)
    b2 = b0
    a0 = f32(1.0) + alpha
    a1 = f32(-2.0) * cos_w0
    a2 = f32(1.0) - alpha
    return (np.float64(b0 / a0), np.float64(b1 / a0), np.float64(b2 / a0),
            np.float64(a1 / a0), np.float64(a2 / a0))


def _impulse_response(coeffs, K):
    b0, b1, b2, a1, a2 = coeffs
    h = np.zeros(K, np.float64)
    y1 = y2 = 0.0
    for n in range(K):
        ff = b0 * (n == 0) + b1 * (n == 1) + b2 * (n == 2)
        y = ff - a1 * y1 - a2 * y2
        h[n] = y
        y2, y1 = y1, y
    return h


def _toeplitz_stationaries(h):
    """stat[k, d*128+m] = h[m - k + 128*d] as the matmul lhsT (stationary)."""
    K = len(h)
    hpad = np.zeros(P * (D + 1), np.float64)
    hpad[:K] = h
    k = np.arange(P)[:, None]
    m = np.arange(P)[None, :]
    blocks = []
    for d in range(D):
        idx = m - k + P * d
        blk = np.where(idx >= 0, hpad[np.clip(idx, 0, None)], 0.0)
        blocks.append(blk)
    return np.concatenate(blocks, axis=1)  # [128, D*128] float64


def _build_module():
    nc = bacc.Bacc(None, target_bir_lowering=False, debug=False)
    f16 = mybir.dt.float16
    f32 = mybir.dt.float32

    xh_d = nc.dram_tensor("xh", [S, TPAD], f16, kind="ExternalInput").ap()
    xl_d = nc.dram_tensor("xl", [S, TPAD], f16, kind="ExternalInput").ap()
    th_d = nc.dram_tensor("th", [P, D * P], f16, kind="ExternalInput").ap()
    tl_d = nc.dram_tensor("tl", [P, D * P], f16, kind="ExternalInput").ap()
    id_d = nc.dram_tensor("ident", [P, P], f32, kind="ExternalInput").ap()
    y_d = nc.dram_tensor("y", [S, TPAD], f32, kind="ExternalOutput").ap()

    with tile.TileContext(nc) as tc:
        with (
            tc.tile_pool(name="const", bufs=1) as const_pool,
            tc.tile_pool(name="xh", bufs=3) as xh_pool,
            tc.tile_pool(name="xl", bufs=3) as xl_pool,
            tc.tile_pool(name="yT", bufs=4) as yT_pool,
            tc.tile_pool(name="ynat", bufs=3) as ynat_pool,
            tc.tile_pool(name="py", bufs=2, space="PSUM") as py_pool,
            tc.tile_pool(name="pt", bufs=6, space="PSUM") as pt_pool,
        ):
            th = const_pool.tile([P, D * P], f16, tag="th")
            tl = const_pool.tile([P, D * P], f16, tag="tl")
            ident = const_pool.tile([P, P], f32, tag="ident")
            nc.sync.dma_start(th[:], th_d[:])
            nc.sync.dma_start(tl[:], tl_d[:])
            nc.sync.dma_start(ident[:], id_d[:])

            def issue_load(s, halves=1):
                # whole-series transposed load (one xbar DMA per half of x;
                # halves=2 splits each into two so compute can start earlier)
                xh = xh_pool.tile([P, HIST + NB], f16, tag="xh")
                xl = xl_pool.tile([P, HIST + NB], f16, tag="xl")
                nc.gpsimd.memset(xh[:, 0:HIST], 0.0)
                nc.gpsimd.memset(xl[:, 0:HIST], 0.0)
                cuts = [0, NB // 2, NB] if halves == 2 else [0, NB]
                for a, b in zip(cuts[:-1], cuts[1:]):
                    nc.sync.dma_start_transpose(
                        xh[:, HIST + a:HIST + b],
                        xh_d[s, a * P:b * P].rearrange("(r c) -> r c", c=P))
                    nc.sync.dma_start_transpose(
                        xl[:, HIST + a:HIST + b],
                        xl_d[s, a * P:b * P].rearrange("(r c) -> r c", c=P))
                return xh, xl

            loads = [issue_load(0, halves=2), issue_load(1, halves=2)]
            for s in range(S):
                xh, xl = loads[s]
                ynat = ynat_pool.tile([P, NB], f32, tag="ynat")
                for g, NG in enumerate(GROUPS):
                    base = 512 * g
                    # Tap blocks d>=2 have |h| small enough that the lo-parts
                    # of both h and x contribute below fp32 noise: hi*hi only.
                    py = py_pool.tile([P, NG], f32, tag="py")
                    passes = []
                    for d in range(D):
                        sl_h = xh[:, HIST + base - d:HIST + base - d + NG]
                        sl_l = xl[:, HIST + base - d:HIST + base - d + NG]
                        st = th[:, d * P:(d + 1) * P]
                        sl = tl[:, d * P:(d + 1) * P]
                        passes.append((st, sl_h))
                        if d < 2:
                            passes.append((sl, sl_h))
                            passes.append((st, sl_l))
                    for i, (w, r) in enumerate(passes):
                        nc.tensor.matmul(py[:], w, r, start=(i == 0),
                                         stop=(i == len(passes) - 1))
                    yT = yT_pool.tile([P, NG], f32, tag="yT")
                    for t2 in range(NG // (2 * P)):
                        nc.scalar.copy(yT[:, t2 * 2 * P:(t2 + 1) * 2 * P],
                                       py[:, t2 * 2 * P:(t2 + 1) * 2 * P])
                    for t in range(NG // P):
                        pt = pt_pool.tile([P, P], f32, tag="pt")
                        nc.tensor.transpose(
                            pt[:], yT[:, t * P:(t + 1) * P], ident[:])
                        nc.vector.tensor_copy(
                            ynat[:, base + t * P:base + (t + 1) * P], pt[:])

                # prefetch the next-next series' load ahead of this series'
                # output DMA so it isn't queued behind the 918KB store
                if s + 2 < S:
                    loads.append(issue_load(s + 2))
                cuts = [0, 896, NB] if s == S - 1 else [0, NB]
                for a, b in zip(cuts[:-1], cuts[1:]):
                    dst = y_d[s, a * P:b * P].rearrange(
                        "(t p c) -> p t c", p=P, c=P)
                    nc.sync.dma_start(
                        dst, ynat[:, a:b].rearrange("p (t c) -> p t c", c=P))
    nc.compile()
    return nc


def _prepare_inputs(audio, sample_rate, cutoff_low, cutoff_high):
    c_lp = _biquad_coeffs("lp", sample_rate, cutoff_low)
    c_hp = _biquad_coeffs("hp", sample_rate, cutoff_high)
    K = P * D
    h = _impulse_response(c_lp, K) - _impulse_response(c_hp, K)
    stat = _toeplitz_stationaries(h)              # [128, D*128] float64
    th = stat.astype(np.float16)
    tl = (stat - th.astype(np.float64)).astype(np.float16)

    x = np.asarray(audio, dtype=np.float32).reshape(S * NCORES, T)
    xpad = np.zeros((S * NCORES, TPAD), np.float32)
    xpad[:, :T] = x
    xh = xpad.astype(np.float16)
    xl = (xpad - xh.astype(np.float32)).astype(np.float16)
    ident = np.eye(P, dtype=np.float32)

    in_maps = []
    for c in range(NCORES):
        in_maps.append({
            "xh": xh[S * c:S * (c + 1)],
            "xl": xl[S * c:S * (c + 1)],
            "th": th,
            "tl": tl,
            "ident": ident,
        })
    return in_maps


def _get_exec():
    """Build the Bass module and a cached sharded jitted executor.

    Returns (sharded_fn, in_names, out_names, out_avals, mesh).  Modeled on
    concourse.bass2jax.run_bass_via_pjrt, but the jitted callable is cached
    so repeated invocations don't re-trace, and timing can target device
    execution only.
    """
    if "exec" in _CACHE:
        return _CACHE["exec"]
    import jax
    from jax.sharding import Mesh, PartitionSpec
    from jax.experimental.shard_map import shard_map
    from concourse import bass2jax as b2j

    nc = _build_module()
    b2j.install_neuronx_cc_hook()

    in_names, out_names, out_avals, zero_outs = [], [], [], []
    partition_name = (nc.partition_id_tensor.name
                      if nc.partition_id_tensor else None)
    for alloc in nc.m.functions[0].allocations:
        if not isinstance(alloc, mybir.MemoryLocationSet):
            continue
        name = alloc.memorylocations[0].name
        if alloc.kind == "ExternalInput":
            if name != partition_name:
                in_names.append(name)
        elif alloc.kind == "ExternalOutput":
            shape = tuple(alloc.tensor_shape)
            dtype = mybir.dt.np(alloc.dtype)
            out_avals.append(jax.core.ShapedArray(shape, dtype))
            out_names.append(name)
            zero_outs.append(np.zeros(shape, dtype))
    n_params = len(in_names)
    n_outs = len(out_avals)
    all_in_names = list(in_names) + list(out_names)
    if partition_name is not None:
        all_in_names.append(partition_name)
    donate = tuple(range(n_params, n_params + n_outs))

    def _body(*args):
        operands = list(args)
        if partition_name is not None:
            operands.append(b2j.partition_id_tensor())
        outs = b2j._bass_exec_p.bind(
            *operands,
            out_avals=tuple(out_avals),
            in_names=tuple(all_in_names),
            out_names=tuple(out_names),
            lowering_input_output_aliases=(),
            sim_require_finite=True,
            sim_require_nnan=True,
            nc=nc,
        )
        return tuple(outs)

    devices = jax.devices()[:NCORES]
    mesh = Mesh(np.asarray(devices), ("core",))
    in_specs = (PartitionSpec("core"),) * (n_params + n_outs)
    out_specs = (PartitionSpec("core"),) * n_outs
    sharded = jax.jit(
        shard_map(_body, mesh=mesh, in_specs=in_specs, out_specs=out_specs,
                  check_rep=False),
        donate_argnums=donate, keep_unused=True)
    _CACHE["exec"] = (sharded, in_names, out_names, out_avals, zero_outs, mesh)
    return _CACHE["exec"]


def _run(audio, sample_rate, cutoff_low, cutoff_high, time_iters=0):
    import jax
    from jax.sharding import NamedSharding, PartitionSpec

    sharded, in_names, out_names, out_avals, zero_outs, mesh = _get_exec()
    in_maps = _prepare_inputs(audio, sample_rate, cutoff_low, cutoff_high)
    concat_in = [
        np.concatenate([np.asarray(in_maps[c][nm]) for c in range(NCORES)],
                       axis=0)
        for nm in in_names
    ]
    concat_zeros = [
        np.zeros((NCORES * z.shape[0], *z.shape[1:]), z.dtype)
        for z in zero_outs
    ]
    sh = NamedSharding(mesh, PartitionSpec("core"))
    dev_in = [jax.device_put(a, sh) for a in concat_in]
    dev_zeros = [jax.device_put(z, sh) for z in concat_zeros]
    out_arrs = sharded(*dev_in, *dev_zeros)
    jax.block_until_ready(out_arrs)

    exec_ns = None
    if time_iters > 0:
        import time
        times = []
        for _ in range(time_iters):
            dz = [jax.device_put(z, sh) for z in concat_zeros]
            jax.block_until_ready(dz)
            t0 = time.perf_counter()
            o = sharded(*dev_in, *dz)
            jax.block_until_ready(o)
            times.append(time.perf_counter() - t0)
        exec_ns = int(min(times) * 1e9)

    iy = out_names.index("y")
    yfull = np.asarray(out_arrs[iy]).reshape(NCORES, S, TPAD)
    out = yfull[:, :, :T].reshape(32, 2, T).astype(np.float32)
    return out, exec_ns


def kernel(audio, sample_rate, cutoff_low, cutoff_high):
    out, _ = _run(audio, sample_rate, cutoff_low, cutoff_high)
    return out

